# Initial kernel scaffold
#
"""EMAStats segment-reduce kernel for 8 Trainium2 NeuronCores (Bass/Tile).

Problem: given logits [B, K], target [B], running (mean, var, count) [K]:
  own[i]     = logits[i, target[i]]
  per class c: n_c = #{i: t_i=c}, s_c = sum own, q_c = sum own^2
  batch_mean = s/n, batch_var = q/n - batch_mean^2
  EMA update with decay 0.1 (first update uses batch stats); classes with
  n_c = 0 keep their buffers.

Strategy (data-parallel over B, 8 cores, B_shard = 16384 rows/core):
  1. Gather, for every row i, the 256-byte block of its logits row that
     contains column target[i], using the production dma_gather (SWDGE)
     instruction -- reads 4MiB of the 128MiB logits shard instead of
     streaming it. dma_gather indexes rows of a [32768, 64] view with
     int16 indices, so the shard is processed as 16 sub-shards of 1024
     rows: idx = local_row*32 + (target>>6) < 32768.
  2. Extract own[i] = block[target[i] & 63] with a 64-wide one-hot and a
     multiply + reduce on the vector engine.
  3. Bin into K = 2048 classes with a deterministic PE-matmul histogram:
     class k = A*16 + B with A in [0,128), B in [0,16). The vector engine
     builds 48 masked-value columns (16 B-groups x {1, own, own^2}) and a
     per-token-tile one-hot over A; 128 accumulating matmuls produce
     psum[A, 48] = sum over tokens. (A scatter-add DMA approach loses
     duplicate-index read-modify-write updates on real HW; indirect DMA
     supports only one offset per partition per instruction.)
  4. AllReduce the [K, 3] partials across the 8 cores.
  5. Apply the EMA update redundantly on every core (K = 2048 is tiny).

Token layout: gather g writes token i (of its 1024) to G[i%128, 8g+i//128, :]
and reads its index from idx16[i%16, 64g+i//16]; with the identity
token->row assignment, position [p, q] (q = 8g+c) holds shard row
elem = g*1024 + c*128 + p. The histogram is order-agnostic, so only
mutual consistency of (G, TL) at each position matters.
"""

import numpy as np

import concourse.bacc as bacc
import concourse.bass as bass
import concourse.mybir as mybir
import concourse.tile as tile
from concourse.bass_utils import run_bass_kernel_spmd

B, K = 131072, 2048
NCORES = 8
BS = B // NCORES  # 16384 rows per core
P = 128
NG = 16  # B-groups (class & 15)
NSUB = 16  # sub-shards for int16 gather indices
SUBR = BS // NSUB  # 1024 rows per sub-shard
BLK = 64  # f32 elements per gathered block (256 bytes)
EMA_DECAY = 0.1
EPS = 1e-12

F32 = mybir.dt.float32
I32 = mybir.dt.int32
I16 = mybir.dt.int16

OP = mybir.AluOpType


def build_program(debug_dumps: bool = False) -> bass.Bass:
    # Bacc (not plain Bass): its compile() lowers the GPSIMD library loads
    # and register allocation that the custom DMA instructions need.
    # 4 SWDGE queues: the 16 sub-shard gathers round-robin across them so
    # Q7 descriptor generation runs on 4 cores concurrently (~3x).
    nc = bacc.Bacc(
        trn_type="TRN2", num_devices=NCORES, debug=False, num_swdge_queues=4
    )

    lg = nc.dram_tensor("logits", [BS * K, 1], F32, kind="ExternalInput")
    tgt = nc.dram_tensor("target", [BS], I32, kind="ExternalInput")
    mean_in = nc.dram_tensor("mean", [K], F32, kind="ExternalInput")
    var_in = nc.dram_tensor("var", [K], F32, kind="ExternalInput")
    cnt_in = nc.dram_tensor("count", [K], I32, kind="ExternalInput")

    new_mean = nc.dram_tensor("new_mean", [K], F32, kind="ExternalOutput")
    new_var = nc.dram_tensor("new_var", [K], F32, kind="ExternalOutput")
    new_count = nc.dram_tensor("new_count", [K], I32, kind="ExternalOutput")

    cc_in = nc.dram_tensor("cc_in", [P, 48], F32)
    cc_out = nc.dram_tensor("cc_out", [P, 48], F32)

    # constants baked into the NEFF: loading them with plain DMAs keeps the
    # GPSIMD queue free so the gathers can start as soon as indices are ready
    ident_c = nc.inline_tensor(np.eye(P, dtype=np.float32), name="ident_c")
    iota_row = np.broadcast_to(np.arange(P, dtype=np.int32), (P, P)).copy()
    iota_t_c = nc.inline_tensor(iota_row, name="iota_t_c")
    iota64_c = nc.inline_tensor(
        np.broadcast_to(np.arange(BLK, dtype=np.int32), (P, BLK)).copy(),
        name="iota64_c",
    )
    sp = np.arange(BS // 16, dtype=np.int32).reshape(NSUB, SUBR // 16) % (SUBR // 16)
    aff_np = (sp[None, :, :] * 16 + np.arange(16, dtype=np.int32)[:, None, None]) * 32
    aff_c = nc.inline_tensor(
        aff_np.reshape(16, BS // 16).astype(np.int32), name="aff_c"
    )

    with tile.TileContext(nc) as tc:
        with (
            tc.tile_pool(name="sb", bufs=1) as sb,
            tc.tile_pool(name="oh", bufs=16) as ohp,
            tc.tile_pool(name="ex", bufs=2) as exp_,
            tc.tile_pool(name="ps", bufs=1, space="PSUM") as ps,
        ):
            # -- gather indices idx = local_row*32 + (target>>6) in the idx
            # wrap TI[p2, 64g+s'] (token s'*16+p2 of gather g at shard row
            # g*1024 + s'*16 + p2), built in 4 pipelined groups of 4
            # sub-shards: strided DRAM load -> shift -> or -> cast ->
            # partition-doubling replicates -> 4 gathers, overlapping the
            # next group's load.
            lgb = lg[:].rearrange("(r e) x -> r (e x)", e=BLK)  # [BS*K/64, 64]
            g_t = sb.tile([P, P * BLK], F32)
            g3 = g_t[:].rearrange("p (q e) -> p q e", e=BLK)
            g3_slices = [g3[:, 8 * g : 8 * (g + 1), :] for g in range(NSUB)]
            aff = sb.tile([16, BS // 16], I32)
            nc.sync.dma_start(out=aff[:], in_=aff_c[:, :])
            idx16 = sb.tile([P, BS // 16], I16)
            GW = BS // 64  # idx cols per group (4 sub-shards)
            for h in range(4):
                gs = slice(GW * h, GW * (h + 1))
                tih = sb.tile([16, GW], I32, name=f"tih_{h}", tag="tih", bufs=2)
                nc.sync.dma_start(
                    out=tih[:, :].rearrange("p (g s) -> p g s", g=4),
                    in_=tgt[4 * h * SUBR : 4 * (h + 1) * SUBR].rearrange(
                        "(g s p) -> p g s", p=16, s=SUBR // 16
                    ),
                )
                blkh = sb.tile([16, GW], I32, name=f"blkh_{h}", tag="blkh", bufs=2)
                nc.vector.tensor_scalar(
                    out=blkh[:], in0=tih[:], scalar1=6, scalar2=None,
                    op0=OP.arith_shift_right,
                )
                idx32h = sb.tile([16, GW], I32, name=f"idx32h_{h}", tag="idx32h", bufs=2)
                nc.vector.tensor_tensor(
                    out=idx32h[:], in0=blkh[:], in1=aff[:, gs], op=OP.bitwise_or
                )
                nc.vector.tensor_copy(out=idx16[:16, gs], in_=idx32h[:])
                for step in range(3):
                    w = 16 << step
                    nc.scalar.dma_start(
                        out=idx16[w : 2 * w, gs], in_=idx16[:w, gs]
                    )
                for g in range(4 * h, 4 * h + 4):
                    nc.gpsimd.dma_gather(
                        g3_slices[g],
                        lgb[g * SUBR * (K // BLK) : (g + 1) * SUBR * (K // BLK), :],
                        idx16[:, (SUBR // 16) * g : (SUBR // 16) * (g + 1)],
                        SUBR,
                        SUBR,
                        elem_size=BLK,
                        queue_num=g % 4,
                    )

            # -- TL[p, q] = target[q*128 + p]: natural contiguous load then a
            # PE transpose (values < 2048 are exact in f32)
            tnat = sb.tile([P, P], I32)
            nc.sync.dma_start(
                out=tnat[:], in_=tgt[:].rearrange("(p f) -> p f", p=P)
            )
            tnatf = sb.tile([P, P], F32)
            nc.vector.tensor_copy(out=tnatf[:], in_=tnat[:])
            ident = sb.tile([P, P], F32)
            nc.scalar.dma_start(out=ident[:], in_=ident_c[:, :])
            ptr = ps.tile([P, P], F32, name="ptr")
            nc.tensor.transpose(out=ptr[:], in_=tnatf[:], identity=ident[:])
            tl = sb.tile([P, P], I32)
            nc.vector.tensor_copy(out=tl[:], in_=ptr[:])

            # -- class decomposition (full tiles, ready early): k = A*16 + B
            a_t = sb.tile([P, P], I32)
            b_t = sb.tile([P, P], I32)
            lowb = sb.tile([P, P], I32)
            nc.vector.tensor_scalar(
                out=a_t[:], in0=tl[:], scalar1=4, scalar2=None,
                op0=OP.arith_shift_right,
            )
            nc.vector.tensor_scalar(
                out=b_t[:], in0=tl[:], scalar1=15, scalar2=None,
                op0=OP.bitwise_and,
            )
            nc.vector.tensor_scalar(
                out=lowb[:], in0=tl[:], scalar1=BLK - 1, scalar2=None,
                op0=OP.bitwise_and,
            )
            iota64s = sb.tile([P, BLK], I32)
            nc.scalar.dma_start(out=iota64s[:], in_=iota64_c[:, :])
            iota_t = sb.tile([P, P], I32)
            nc.scalar.dma_start(out=iota_t[:], in_=iota_t_c[:, :])

            # -- A-one-hot tiles (need only a_t): first 4 up front so the PE
            # matmul chain can start, the rest after the extraction emissions
            oh8s = [None] * NSUB

            def build_oh8(j):
                oh8 = ohp.tile([P, 8 * P], F32, name=f"oh8_{j}", tag="oh8")
                nc.vector.tensor_tensor(
                    out=oh8[:].rearrange("p (c a) -> p c a", a=P),
                    in0=a_t[:, 8 * j : 8 * (j + 1)][:, :, None].to_broadcast(
                        [P, 8, P]
                    ),
                    in1=iota_t[:, None, :].to_broadcast([P, 8, P]),
                    op=OP.is_equal,
                )
                oh8s[j] = oh8

            for j in range(4):
                build_oh8(j)


            # -- per-quarter (32 token cols): extract own values, build the
            # 48 masked-value columns; overlaps with later gathers
            v = sb.tile([P, P], F32)
            vmall = sb.tile([P, P * NG * 3], F32)
            vm4 = vmall[:].rearrange("p (c g s) -> p c g s", g=NG, s=3)
            # count-stat mask columns need only b_t -- build them now so only
            # the two value products remain on the post-extraction path
            for g in range(NG):
                nc.vector.tensor_scalar(
                    out=vm4[:, :, g, 0], in0=b_t[:], scalar1=float(g),
                    scalar2=None, op0=OP.is_equal,
                )
            QT = P // 4
            for qt in range(4):
                cs = slice(QT * qt, QT * (qt + 1))
                ohq = exp_.tile([P, QT * BLK], F32, name=f"ohq_{qt}", tag="ohq")
                ohq3 = ohq[:].rearrange("p (q e) -> p q e", e=BLK)
                nc.vector.tensor_tensor(
                    out=ohq3[:],
                    in0=lowb[:, cs][:, :, None].to_broadcast([P, QT, BLK]),
                    in1=iota64s[:, None, :].to_broadcast([P, QT, BLK]),
                    op=OP.is_equal,
                )
                nc.vector.tensor_tensor(
                    out=ohq3[:], in0=g3[:, cs, :], in1=ohq3[:],
                    op=OP.mult,
                )
                nc.vector.tensor_reduce(
                    out=v[:, cs],
                    in_=ohq3[:],
                    axis=mybir.AxisListType.X,
                    op=OP.add,
                )
                # value products for this quarter (masks already built)
                for g in range(NG):
                    nc.vector.scalar_tensor_tensor(
                        out=vm4[:, cs, g, 1], in0=b_t[:, cs],
                        scalar=float(g), in1=v[:, cs],
                        op0=OP.is_equal, op1=OP.mult,
                    )
                    nc.vector.tensor_tensor(
                        out=vm4[:, cs, g, 2], in0=vm4[:, cs, g, 1],
                        in1=v[:, cs], op=OP.mult,
                    )
                for j in range(4 + 3 * qt, 4 + 3 * (qt + 1)):
                    if j < NSUB and oh8s[j] is None:
                        build_oh8(j)

            if debug_dumps:
                dbg_v = nc.dram_tensor("dbg_v", [P, P], F32, kind="ExternalOutput")
                dbg_tl = nc.dram_tensor("dbg_tl", [P, P], I32, kind="ExternalOutput")
                nc.sync.dma_start(out=dbg_v[:, :], in_=v[:])
                nc.sync.dma_start(out=dbg_tl[:, :], in_=tl[:])

            # -- histogram: psum[A, 48] += onehot_c^T @ vmall_c over 128 token
            # cols; one-hots built 8 columns per DVE op
            for j in range(NSUB):
                if oh8s[j] is None:
                    build_oh8(j)

            pstats = ps.tile([P, 48], F32)
            for j in range(NSUB):
                for cc in range(8):
                    c = 8 * j + cc
                    nc.tensor.matmul(
                        out=pstats[:],
                        lhsT=oh8s[j][:, P * cc : P * (cc + 1)],
                        rhs=vm4[:, c, :, :],
                        start=(c == 0),
                        stop=(c == P - 1),
                    )

            # local partials [128, 48] = [A, grp, s]; class k = A*16 + grp
            st = sb.tile([P, 48], F32)
            nc.vector.tensor_copy(out=st[:], in_=pstats[:])

            # -- all-reduce partials across the 8 cores
            nc.sync.dma_start(out=cc_in[:, :], in_=st[:])
            nc.gpsimd.collective_compute(
                "AllReduce",
                OP.add,
                replica_groups=[list(range(NCORES))],
                ins=[cc_in.ap().opt()],
                outs=[cc_out.ap().opt()],
            )
            stg = sb.tile([P, 48], F32)
            nc.sync.dma_start(out=stg[:], in_=cc_out[:, :])
            stg3 = stg[:].rearrange("p (c s) -> p c s", s=3)

            # -- EMA update on [128, 16] tiles (class = p*16 + chunk)
            _t16_id = [0]

            def t16f(dtype=F32):
                _t16_id[0] += 1
                return sb.tile([P, NG], dtype, name=f"t16_{_t16_id[0]}")

            n_t, s_t, q_t = t16f(), t16f(), t16f()
            nc.vector.tensor_copy(out=n_t[:], in_=stg3[:, :, 0])
            nc.vector.tensor_copy(out=s_t[:], in_=stg3[:, :, 1])
            nc.vector.tensor_copy(out=q_t[:], in_=stg3[:, :, 2])

            m_t, va_t, c_t = t16f(), t16f(), t16f(I32)
            nc.sync.dma_start(out=m_t[:], in_=mean_in[:].rearrange("(p c) -> p c", p=P))
            nc.sync.dma_start(out=va_t[:], in_=var_in[:].rearrange("(p c) -> p c", p=P))
            nc.sync.dma_start(out=c_t[:], in_=cnt_in[:].rearrange("(p c) -> p c", p=P))

            ns_t, rn_t, bm_t, bv_t = t16f(), t16f(), t16f(), t16f()
            nc.vector.tensor_scalar_max(out=ns_t[:], in0=n_t[:], scalar1=1.0)
            nc.vector.reciprocal(out=rn_t[:], in_=ns_t[:])
            nc.vector.tensor_tensor(out=bm_t[:], in0=s_t[:], in1=rn_t[:], op=OP.mult)
            # bv = q/n - bm^2
            qn_t, bm2_t = t16f(), t16f()
            nc.vector.tensor_tensor(out=qn_t[:], in0=q_t[:], in1=rn_t[:], op=OP.mult)
            nc.vector.tensor_tensor(out=bm2_t[:], in0=bm_t[:], in1=bm_t[:], op=OP.mult)
            nc.vector.tensor_tensor(
                out=bv_t[:], in0=qn_t[:], in1=bm2_t[:], op=OP.subtract
            )

            # masks: first = (count == 0), has = (n > 0); uint8 0/1
            # (CopyPredicated requires an integer mask dtype)
            cf_t = t16f()
            first_t, has_t = t16f(mybir.dt.uint8), t16f(mybir.dt.uint8)
            nc.vector.tensor_copy(out=cf_t[:], in_=c_t[:])
            nc.vector.tensor_scalar(
                out=first_t[:], in0=cf_t[:], scalar1=0.0, scalar2=None,
                op0=OP.is_equal,
            )
            nc.vector.tensor_scalar(
                out=has_t[:], in0=n_t[:], scalar1=0.0, scalar2=None, op0=OP.is_gt
            )

            # em = mean + decay*(bm - mean); ev = var + decay*(bv - var)
            d_t, em_t, ev_t = t16f(), t16f(), t16f()
            nc.vector.tensor_tensor(out=d_t[:], in0=bm_t[:], in1=m_t[:], op=OP.subtract)
            nc.vector.scalar_tensor_tensor(
                out=em_t[:], in0=d_t[:], scalar=EMA_DECAY, in1=m_t[:],
                op0=OP.mult, op1=OP.add,
            )
            nc.vector.tensor_tensor(
                out=d_t[:], in0=bv_t[:], in1=va_t[:], op=OP.subtract
            )
            nc.vector.scalar_tensor_tensor(
                out=ev_t[:], in0=d_t[:], scalar=EMA_DECAY, in1=va_t[:],
                op0=OP.mult, op1=OP.add,
            )

            # cand_mean = first ? bm : em ; cand_var = max(first ? bv : ev, EPS)
            cm_t, cv_t = t16f(), t16f()
            nc.vector.select(out=cm_t[:], mask=first_t[:], on_true=bm_t[:], on_false=em_t[:])
            nc.vector.select(out=cv_t[:], mask=first_t[:], on_true=bv_t[:], on_false=ev_t[:])
            nc.vector.tensor_scalar_max(out=cv_t[:], in0=cv_t[:], scalar1=EPS)

            # new_mean/var = has ? cand : old ; new_count = count + n
            nm_t, nv_t = t16f(), t16f()
            nc.vector.select(out=nm_t[:], mask=has_t[:], on_true=cm_t[:], on_false=m_t[:])
            nc.vector.select(out=nv_t[:], mask=has_t[:], on_true=cv_t[:], on_false=va_t[:])
            ni_t, ncnt_t = t16f(I32), t16f(I32)
            nc.vector.tensor_copy(out=ni_t[:], in_=n_t[:])
            nc.vector.tensor_tensor(out=ncnt_t[:], in0=c_t[:], in1=ni_t[:], op=OP.add)

            nc.sync.dma_start(
                out=new_mean[:].rearrange("(p c) -> p c", p=P), in_=nm_t[:]
            )
            nc.sync.dma_start(
                out=new_var[:].rearrange("(p c) -> p c", p=P), in_=nv_t[:]
            )
            nc.sync.dma_start(
                out=new_count[:].rearrange("(p c) -> p c", p=P), in_=ncnt_t[:]
            )

    nc.compile()
    return nc


def make_in_maps(logits, target, mean, var, count):
    """Shard the full inputs into per-core input maps."""
    logits = np.ascontiguousarray(np.asarray(logits, dtype=np.float32))
    target = np.asarray(target).astype(np.int32)
    mean = np.asarray(mean, dtype=np.float32)
    var = np.asarray(var, dtype=np.float32)
    count_i32 = np.asarray(count).astype(np.int32)

    in_maps = []
    for m in range(NCORES):
        rows = slice(m * BS, (m + 1) * BS)
        in_maps.append(
            {
                "logits": logits[rows].reshape(BS * K, 1),
                "target": target[rows],
                "mean": mean,
                "var": var,
                "count": count_i32,
            }
        )
    return in_maps


_NC_CACHE = None


def kernel(logits, target, mean, var, count):
    global _NC_CACHE
    if _NC_CACHE is None:
        _NC_CACHE = build_program()
    nc = _NC_CACHE

    in_maps = make_in_maps(logits, target, mean, var, count)
    res = run_bass_kernel_spmd(nc, in_maps, list(range(NCORES)))
    out = res.results[0]

    count_dtype = np.asarray(count).dtype
    return (
        out["new_mean"].reshape(K).astype(np.float32),
        out["new_var"].reshape(K).astype(np.float32),
        out["new_count"].reshape(K).astype(count_dtype),
    )



# revision 7
# speedup vs baseline: 1.5609x; 1.5609x over previous
"""EMAStats segment-reduce kernel for 8 Trainium2 NeuronCores (Bass/Tile).

Problem: given logits [B, K], target [B], running (mean, var, count) [K]:
  own[i]     = logits[i, target[i]]
  per class c: n_c = #{i: t_i=c}, s_c = sum own, q_c = sum own^2
  batch_mean = s/n, batch_var = q/n - batch_mean^2
  EMA update with decay 0.1 (first update uses batch stats); classes with
  n_c = 0 keep their buffers.

Strategy (data-parallel over B, 8 cores, BS = 16384 rows/core):
  1. dma_gather (SWDGE, 16 gathers of 1024 blocks on 4 queues) fetches, for
     every row, the 256-byte block of its logits row containing column
     target[i]. Block index = target>>6, which IS the high part A2 of the
     class decomposition k = A2*64 + e (A2 in [0,32), e = k&63).
  2. The gather indices are derived from ONE contiguous [16, 1024] load of
     target (256 descriptors) by choosing the slot->row permutation
     sigma_g(j) = 64*(j%16) + j//16, so idx16 = (L>>6)|aff is built in place
     with two tiny DVE ops -- the gathers start ~4us into the kernel.
  3. Fused extraction+histogram: per 128-token column q, rhs[p, 0:64] =
     maskE = [lowb==e] (count cols), rhs[p, 64:128] = maskE*G (value cols),
     rhs[p, 128:192] = (maskE*G)^2 (squares, on the scalar engine); lhsT =
     onehot over A2 (32 wide). One f32r matmul per token column accumulates
     psum[A2, 256] over all 128 columns: psum[A2, e]=n, [A2,64+e]=s,
     [A2,128+e]=q for class k = A2*64+e. maskE doubles as the extraction
     one-hot AND the count column, so no separate own-extraction pass, and
     f32r at 256 moving cols streams 1 cycle/row (vs fp32's 4).
  4. AllReduce of the [32, 192] partials via a 3-round XOR butterfly of
     remote_dma_broadcast (one real rdest (0, 1<<r) per round, slot chosen
     for the D2D rule), with gpsimd adds -- replaces the ~30us CC AllReduce.
  5. EMA update redundantly on every core on [32, 64] tiles.
"""

import numpy as np

import concourse.bacc as bacc
import concourse.bass as bass
import concourse.mybir as mybir
import concourse.tile as tile
from concourse.bass_utils import run_bass_kernel_spmd

B, K = 131072, 2048
NCORES = 8
BS = B // NCORES  # 16384 rows per core
P = 128
NSUB = 16  # gathers (sub-shards of 1024 rows; int16 idx = row*32+blk < 32768)
SUBR = BS // NSUB  # 1024
BLK = 64  # f32 elements per gathered block (256 bytes)
A2W = K // BLK  # 32: one-hot width of k>>6 (psum partition dim)
RW = 192  # rhs width: 64 mask | 64 val | 64 sq (fp16: 1 cyc/row on PE)
GPG = 4  # gathers per pipeline group
NGRP = NSUB // GPG  # 4
QT = 8 * GPG  # token columns per group = 32
EMA_DECAY = 0.1
EPS = 1e-12
USE_CC = True  # fallback: CC AllReduce instead of the remote-DMA butterfly

F32 = mybir.dt.float32
F16 = mybir.dt.float16
I32 = mybir.dt.int32
I16 = mybir.dt.int16

OP = mybir.AluOpType


def build_program() -> bass.Bass:
    nc = bacc.Bacc(
        trn_type="TRN2", num_devices=NCORES, debug=False, num_swdge_queues=4
    )

    lg = nc.dram_tensor("logits", [BS * K, 1], F32, kind="ExternalInput")
    tgt = nc.dram_tensor("target", [BS], I32, kind="ExternalInput")
    mean_in = nc.dram_tensor("mean", [K], F32, kind="ExternalInput")
    var_in = nc.dram_tensor("var", [K], F32, kind="ExternalInput")
    cnt_in = nc.dram_tensor("count", [K], I32, kind="ExternalInput")

    new_mean = nc.dram_tensor("new_mean", [K], F32, kind="ExternalOutput")
    new_var = nc.dram_tensor("new_var", [K], F32, kind="ExternalOutput")
    new_count = nc.dram_tensor("new_count", [K], I32, kind="ExternalOutput")

    if USE_CC:
        cc_in = nc.dram_tensor("cc_in", [A2W, 192], F32)
        cc_out = nc.dram_tensor("cc_out", [A2W, 192], F32)

    # constants
    cols = np.arange(BS // 16, dtype=np.int32)
    aff_np = 2048 * np.arange(16, dtype=np.int32)[:, None] + 32 * (cols % BLK)[None, :]
    aff_c = nc.inline_tensor(aff_np, name="aff_c")
    iota64_c = nc.inline_tensor(
        np.broadcast_to(np.arange(BLK, dtype=np.int32), (P, BLK)).copy(),
        name="iota64_c",
    )
    iota32_c = nc.inline_tensor(
        np.broadcast_to(np.arange(A2W, dtype=np.int32), (P, A2W)).copy(),
        name="iota32_c",
    )

    with tile.TileContext(nc) as tc:
        with (
            tc.tile_pool(name="sb", bufs=1) as sb,
            tc.tile_pool(name="rh", bufs=2) as rh,
            tc.tile_pool(name="ps", bufs=1, space="PSUM") as ps,
        ):
            # -- L[p2, 64g+u] = target[1024g + 64*p2 + u]: one strided load,
            # 256 descriptors of 256B. Gather g slot j fetches local row
            # sigma_g(j) = 64*(j%16) + j//16, so idx16 = (L>>6)|aff in place.
            L = sb.tile([16, BS // 16], I32)
            nc.sync.dma_start(
                out=L[:].rearrange("p (g u) -> p g u", u=BLK),
                in_=tgt[:].rearrange("(g p u) -> p g u", p=16, u=BLK),
            )
            aff = sb.tile([16, BS // 16], I32)
            nc.scalar.dma_start(out=aff[:], in_=aff_c[:, :])
            blk_t = sb.tile([16, BS // 16], I32)
            nc.vector.tensor_scalar(
                out=blk_t[:], in0=L[:], scalar1=6, scalar2=None,
                op0=OP.arith_shift_right,
            )
            idx32 = sb.tile([16, BS // 16], I32)
            nc.vector.tensor_tensor(
                out=idx32[:], in0=blk_t[:], in1=aff[:], op=OP.bitwise_or
            )
            idx16 = sb.tile([P, BS // 16], I16)
            nc.vector.tensor_copy(out=idx16[:16, :], in_=idx32[:])
            for step in range(3):
                w = 16 << step
                nc.scalar.dma_start(out=idx16[w : 2 * w, :], in_=idx16[:w, :])

            # -- 16 gathers; gather g writes slot j to G[j%128, 8g+j//128, :]
            lgb = lg[:].rearrange("(r e) x -> r (e x)", e=BLK)  # [BS*32, 64]
            g_t = sb.tile([P, P * BLK], F32)
            g3 = g_t[:].rearrange("p (q e) -> p q e", e=BLK)
            for g in range(NSUB):
                nc.gpsimd.dma_gather(
                    g3[:, 8 * g : 8 * (g + 1), :],
                    lgb[g * SUBR * (K // BLK) : (g + 1) * SUBR * (K // BLK), :],
                    idx16[:, 64 * g : 64 * (g + 1)],
                    SUBR,
                    SUBR,
                    elem_size=BLK,
                    queue_num=g % 4,
                )

            # -- TL[p, q] = target of token (p, q) = L[p&15, 8q + (p>>4)]:
            # engine ops can't start at partition 16b, so de-interleave the 8
            # bands at partition offset 0 into a staging tile (8 legal DVE
            # copies), then 8 tiny SBUF->SBUF DMAs (DMAs cross partitions
            # freely) place band b at partitions [16b, 16b+16).
            stage = sb.tile([16, 8 * P], I32)
            L3 = L[:].rearrange("p (q b) -> p q b", b=8)
            for b in range(8):
                nc.vector.tensor_copy(
                    out=stage[:, P * b : P * (b + 1)], in_=L3[:, :, b]
                )
            tl = sb.tile([P, P], I32)
            for b in range(8):
                nc.scalar.dma_start(
                    out=tl[16 * b : 16 * (b + 1), :],
                    in_=stage[:, P * b : P * (b + 1)],
                )
            a2_t = sb.tile([P, P], I32)
            lowb = sb.tile([P, P], I32)
            nc.vector.tensor_scalar(
                out=a2_t[:], in0=tl[:], scalar1=6, scalar2=None,
                op0=OP.arith_shift_right,
            )
            nc.vector.tensor_scalar(
                out=lowb[:], in0=tl[:], scalar1=BLK - 1, scalar2=None,
                op0=OP.bitwise_and,
            )
            iota64s = sb.tile([P, BLK], I32)
            nc.scalar.dma_start(out=iota64s[:], in_=iota64_c[:, :])
            iota32s = sb.tile([P, A2W], I32)
            nc.scalar.dma_start(out=iota32s[:], in_=iota32_c[:, :])

            # -- fused extraction+histogram, pipelined in NGRP groups
            pstats = ps.tile([A2W, RW], F32, name="pstats")
            for h in range(NGRP):
                qs = slice(QT * h, QT * (h + 1))
                rhs = rh.tile([P, QT * RW], F16, name=f"rhs_{h}", tag="rhs")
                rhs3 = rhs[:].rearrange("p (q w) -> p q w", w=RW)
                a2oh = rh.tile([P, QT * A2W], F16, name=f"a2oh_{h}", tag="a2oh")
                a2oh3 = a2oh[:].rearrange("p (q a) -> p q a", a=A2W)
                # maskE = [lowb == e]: count cols AND the extraction one-hot
                nc.vector.tensor_tensor(
                    out=rhs3[:, :, 0:BLK],
                    in0=lowb[:, qs][:, :, None].to_broadcast([P, QT, BLK]),
                    in1=iota64s[:, None, :].to_broadcast([P, QT, BLK]),
                    op=OP.is_equal,
                )
                # value cols = maskE * G
                nc.vector.tensor_tensor(
                    out=rhs3[:, :, BLK : 2 * BLK],
                    in0=rhs3[:, :, 0:BLK],
                    in1=g3[:, qs, :],
                    op=OP.mult,
                )
                # square cols on the scalar engine (off the DVE critical path)
                nc.scalar.square(
                    out=rhs3[:, :, 2 * BLK : 3 * BLK],
                    in_=rhs3[:, :, BLK : 2 * BLK],
                )
                # A2 one-hot (lhsT)
                nc.vector.tensor_tensor(
                    out=a2oh3[:],
                    in0=a2_t[:, qs][:, :, None].to_broadcast([P, QT, A2W]),
                    in1=iota32s[:, None, :].to_broadcast([P, QT, A2W]),
                    op=OP.is_equal,
                )
                for c in range(QT):
                    cg = QT * h + c
                    nc.tensor.matmul(
                        out=pstats[:],
                        lhsT=a2oh3[:, c, :],
                        rhs=rhs3[:, c, :],
                        start=(cg == 0),
                        stop=(cg == P - 1),
                    )

            # -- local partials [32, 192]: n | s | q for class k = A2*64 + e
            st = sb.tile([P, 192], F32, name="bsend0")
            nc.vector.tensor_copy(out=st[:A2W, :], in_=pstats[:, 0:192])

            # -- all-reduce across the 8 cores
            if USE_CC:
                nc.sync.dma_start(out=cc_in[:, :], in_=st[:A2W, :])
                nc.gpsimd.collective_compute(
                    "AllReduce",
                    OP.add,
                    replica_groups=[list(range(NCORES))],
                    ins=[cc_in.ap().opt()],
                    outs=[cc_out.ap().opt()],
                )
                acc = sb.tile([P, 192], F32, name="bacc")
                nc.sync.dma_start(out=acc[:A2W, :], in_=cc_out[:, :])
            else:
                # 3-round XOR butterfly: round r exchanges with peer id^(1<<r)
                # (remote_dma_broadcast with a single real relative dest) and
                # adds. Sems/adds all on gpsimd: engine FIFO inside
                # tile_critical orders wait -> add; recv tiles are dedicated.
                bsend = [st]
                for r in (1, 2):
                    bsend.append(sb.tile([P, 192], F32, name=f"bsend{r}"))
                acc = sb.tile([P, 192], F32, name="bacc")
                brecv = [
                    sb.tile([P, 192], F32, name=f"brecv{r}") for r in range(3)
                ]
                rsem = [nc.alloc_semaphore(f"bf_r{r}") for r in range(3)]
                lsem = [nc.alloc_semaphore(f"bf_l{r}") for r in range(3)]
                with tc.tile_critical(name="bfly"):
                    for r in range(3):
                        d = 1 << r
                        rd: list = [None] * 8
                        # cross-die dests (bit 2 of dtpb) must sit in slot 4-7
                        rd[4 if d & 4 else 0] = (0, d)
                        nc.gpsimd.remote_dma_broadcast(
                            out_ap=brecv[r][:],
                            in_ap=bsend[r][:],
                            remote_sem=rsem[r],
                            local_sem=lsem[r],
                            rdests=rd,
                        )
                        nc.gpsimd.trigger_dma(count=None)
                        nc.gpsimd.wait_ge(rsem[r], 2)
                        dst = bsend[r + 1] if r < 2 else acc
                        nc.gpsimd.tensor_tensor(
                            out=dst[:A2W, :],
                            in0=bsend[r][:A2W, :],
                            in1=brecv[r][:A2W, :],
                            op=OP.add,
                        )

            n_t = acc[:A2W, 0:64]
            s_t = acc[:A2W, 64:128]
            q_t = acc[:A2W, 128:192]

            # -- EMA update on [32, 64] tiles (class k = p*64 + e)
            _tid = [0]

            def t64(dtype=F32):
                _tid[0] += 1
                return sb.tile([A2W, BLK], dtype, name=f"t64_{_tid[0]}")

            m_t, va_t, c_t = t64(), t64(), t64(I32)
            nc.sync.dma_start(out=m_t[:], in_=mean_in[:].rearrange("(p c) -> p c", p=A2W))
            nc.sync.dma_start(out=va_t[:], in_=var_in[:].rearrange("(p c) -> p c", p=A2W))
            nc.sync.dma_start(out=c_t[:], in_=cnt_in[:].rearrange("(p c) -> p c", p=A2W))

            ns_t, rn_t, bm_t, bv_t = t64(), t64(), t64(), t64()
            nc.vector.tensor_scalar_max(out=ns_t[:], in0=n_t, scalar1=1.0)
            nc.vector.reciprocal(out=rn_t[:], in_=ns_t[:])
            nc.vector.tensor_tensor(out=bm_t[:], in0=s_t, in1=rn_t[:], op=OP.mult)
            qn_t, bm2_t = t64(), t64()
            nc.vector.tensor_tensor(out=qn_t[:], in0=q_t, in1=rn_t[:], op=OP.mult)
            nc.vector.tensor_tensor(out=bm2_t[:], in0=bm_t[:], in1=bm_t[:], op=OP.mult)
            nc.vector.tensor_tensor(
                out=bv_t[:], in0=qn_t[:], in1=bm2_t[:], op=OP.subtract
            )

            # masks: first = (count == 0), has = (n > 0); uint8 0/1
            cf_t = t64()
            first_t, has_t = t64(mybir.dt.uint8), t64(mybir.dt.uint8)
            nc.vector.tensor_copy(out=cf_t[:], in_=c_t[:])
            nc.vector.tensor_scalar(
                out=first_t[:], in0=cf_t[:], scalar1=0.0, scalar2=None,
                op0=OP.is_equal,
            )
            nc.vector.tensor_scalar(
                out=has_t[:], in0=n_t, scalar1=0.0, scalar2=None, op0=OP.is_gt
            )

            # em = mean + decay*(bm - mean); ev = var + decay*(bv - var)
            d_t, em_t, ev_t = t64(), t64(), t64()
            nc.vector.tensor_tensor(out=d_t[:], in0=bm_t[:], in1=m_t[:], op=OP.subtract)
            nc.vector.scalar_tensor_tensor(
                out=em_t[:], in0=d_t[:], scalar=EMA_DECAY, in1=m_t[:],
                op0=OP.mult, op1=OP.add,
            )
            nc.vector.tensor_tensor(
                out=d_t[:], in0=bv_t[:], in1=va_t[:], op=OP.subtract
            )
            nc.vector.scalar_tensor_tensor(
                out=ev_t[:], in0=d_t[:], scalar=EMA_DECAY, in1=va_t[:],
                op0=OP.mult, op1=OP.add,
            )

            cm_t, cv_t = t64(), t64()
            nc.vector.select(out=cm_t[:], mask=first_t[:], on_true=bm_t[:], on_false=em_t[:])
            nc.vector.select(out=cv_t[:], mask=first_t[:], on_true=bv_t[:], on_false=ev_t[:])
            nc.vector.tensor_scalar_max(out=cv_t[:], in0=cv_t[:], scalar1=EPS)

            nm_t, nv_t = t64(), t64()
            nc.vector.select(out=nm_t[:], mask=has_t[:], on_true=cm_t[:], on_false=m_t[:])
            nc.vector.select(out=nv_t[:], mask=has_t[:], on_true=cv_t[:], on_false=va_t[:])
            ni_t, ncnt_t = t64(I32), t64(I32)
            nc.vector.tensor_copy(out=ni_t[:], in_=n_t)
            nc.vector.tensor_tensor(out=ncnt_t[:], in0=c_t[:], in1=ni_t[:], op=OP.add)

            nc.sync.dma_start(
                out=new_mean[:].rearrange("(p c) -> p c", p=A2W), in_=nm_t[:]
            )
            nc.sync.dma_start(
                out=new_var[:].rearrange("(p c) -> p c", p=A2W), in_=nv_t[:]
            )
            nc.sync.dma_start(
                out=new_count[:].rearrange("(p c) -> p c", p=A2W), in_=ncnt_t[:]
            )

    nc.compile()
    return nc


def make_in_maps(logits, target, mean, var, count):
    """Shard the full inputs into per-core input maps."""
    logits = np.ascontiguousarray(np.asarray(logits, dtype=np.float32))
    target = np.asarray(target).astype(np.int32)
    mean = np.asarray(mean, dtype=np.float32)
    var = np.asarray(var, dtype=np.float32)
    count_i32 = np.asarray(count).astype(np.int32)

    in_maps = []
    for m in range(NCORES):
        rows = slice(m * BS, (m + 1) * BS)
        in_maps.append(
            {
                "logits": logits[rows].reshape(BS * K, 1),
                "target": target[rows],
                "mean": mean,
                "var": var,
                "count": count_i32,
            }
        )
    return in_maps


_NC_CACHE = None


def kernel(logits, target, mean, var, count):
    global _NC_CACHE
    if _NC_CACHE is None:
        _NC_CACHE = build_program()
    nc = _NC_CACHE

    in_maps = make_in_maps(logits, target, mean, var, count)
    res = run_bass_kernel_spmd(nc, in_maps, list(range(NCORES)))
    out = res.results[0]

    count_dtype = np.asarray(count).dtype
    return (
        out["new_mean"].reshape(K).astype(np.float32),
        out["new_var"].reshape(K).astype(np.float32),
        out["new_count"].reshape(K).astype(count_dtype),
    )
